# revision 1
# baseline (speedup 1.0000x reference)
"""Trainium2 Bass kernel for nn_DiffusionDynamicOutput.

Math (per batch b, one NeuronCore each):
  ctx  = wv_embs[b] + t_emb[b]            (N=32, E=1024)
  h    = silu(ctx @ W1 + b1)              (32, 256)
  w    = h @ W2 + b2                      (32, 576) -> (N, D=64, K*K=9), k = di*3+dj
  out[n,h,w] = sum_{d,di,dj} w[n,d,di*3+dj] * latent[d, h+di-1, w+dj-1]  (zero-pad)

Kernel strategy per core (v4):
  - t_emb@W1 folded into the layer-1 bias; ctx^T = wv^T via PE transposes.
  - latent in 4 overlapping column-segment tiles (64, SEGW); h-shifts (di)
    are +/-W column offsets, so each 512-col psum tile takes 3 accumulating
    C=64 matmuls (float32r, full rate at F=512): M=(dj,n)=96 -> psum rows
    0:32 = dj0 raw, 32:64 = dj1 center, 64:96 = dj2 raw.
  - f-tiles processed in groups of GSIZE sharing one multi-bank PSUM tile;
    the w-shift (dj) combine runs batched: one ACT center copy + two DVE
    shifted adds per group (amortizes per-op overheads).
"""

import numpy as np

import concourse.bass as bass
import concourse.tile as tile
from concourse import bacc, mybir
from concourse.bass_utils import run_bass_kernel_spmd
from concourse.masks import make_identity

F32 = mybir.dt.float32
F32R = mybir.dt.float32r
AFT = mybir.ActivationFunctionType

B, N, D, H, W = 8, 32, 64, 128, 128
HW = H * W  # 16384
E, HID, KK = 1024, 256, 9
FT = 512  # columns per PSUM bank tile (4 image rows)
NFT = HW // FT  # 32
GSIZE = 2  # f-tiles per psum mega-tile (banks)
NG = NFT // GSIZE  # 8 groups

NSEG = 8
SEGF = NFT // NSEG
SEGC = SEGF * FT  # 4096
MARG = W  # margin cols each side for the h-shifts
SEGW = MARG + SEGC + MARG


def build_nc(n_cores: int = 8):
    nc = bacc.Bacc(
        "TRN2",
        target_bir_lowering=False,
        debug=False,
        enable_asserts=False,
        num_devices=n_cores,
    )
    lat = nc.dram_tensor("lat", [D, HW], F32, kind="ExternalInput").ap()
    temb = nc.dram_tensor("temb", [E], F32, kind="ExternalInput").ap()
    wv = nc.dram_tensor("wv", [N, E], F32, kind="ExternalInput").ap()
    w1 = nc.dram_tensor("w1", [E, HID], F32, kind="ExternalInput").ap()
    b1 = nc.dram_tensor("b1", [HID], F32, kind="ExternalInput").ap()
    w2 = nc.dram_tensor("w2", [HID, D * KK], F32, kind="ExternalInput").ap()
    b2 = nc.dram_tensor("b2", [D * KK], F32, kind="ExternalInput").ap()
    out = nc.dram_tensor("out", [N, HW], F32, kind="ExternalOutput").ap()

    with tile.TileContext(nc) as tc:
        _emit(tc, lat, temb, wv, w1, b1, w2, b2, out)
    nc.compile()
    return nc


def _emit(tc, lat, temb, wv, w1, b1, w2, b2, out):
    from contextlib import ExitStack

    nc = tc.nc
    ctx = ExitStack()
    singles = ctx.enter_context(tc.tile_pool(name="singles", bufs=1))
    psP = ctx.enter_context(tc.tile_pool(name="psP", bufs=4, space="PSUM"))
    outP = ctx.enter_context(tc.tile_pool(name="outP", bufs=6))
    try:
        _emit_body(nc, singles, psP, outP, lat, temb, wv, w1, b1, w2, b2, out)
    finally:
        ctx.close()


def _emit_body(nc, singles, psP, outP, lat, temb, wv, w1, b1, w2, b2, out):
    # ---- static params into SBUF (wv first: transposes gate the MLP) ----
    wv_sb = singles.tile([N, E], F32)
    nc.sync.dma_start(out=wv_sb, in_=wv)
    w1_sb = singles.tile([128, E // 128, HID], F32)
    nc.sync.dma_start(out=w1_sb, in_=w1.rearrange("(c p) h -> p c h", p=128))
    w2_sb = singles.tile([128, HID // 128, D * KK], F32)
    nc.sync.dma_start(out=w2_sb, in_=w2.rearrange("(c p) o -> p c o", p=128))
    b1_sb = singles.tile([128, HID // 128], F32)
    nc.sync.dma_start(out=b1_sb, in_=b1.rearrange("(m p) -> p m", p=128))
    temb_sb = singles.tile([128, E // 128], F32)
    nc.sync.dma_start(out=temb_sb, in_=temb.rearrange("(c p) -> p c", p=128))
    b2_sb = singles.tile([D, KK], F32)
    nc.sync.dma_start(out=b2_sb, in_=b2.rearrange("(d k) -> d k", k=KK))

    ident = singles.tile([N, N], F32)
    make_identity(nc, ident)

    EC = E // 128  # 8
    C2 = HID // 128  # 2

    # ---- ctx^T = wv^T via PE transposes (first PE work; only needs wv) ----
    ctxT = singles.tile([128, EC, N], F32)
    for c in range(EC):
        tp = psP.tile([128, N], F32, tag="mega")
        nc.tensor.transpose(tp, wv_sb[:, c * 128 : (c + 1) * 128], ident)
        nc.scalar.copy(out=ctxT[:, c, :], in_=tp)

    # ---- layer 1 matmuls ----
    hpsum = []
    for m in range(C2):
        hp = psP.tile([128, N], F32, tag="mega", name=f"hp{m}")
        hpsum.append(hp)
        for c in range(EC):
            nc.tensor.matmul(
                hp,
                lhsT=w1_sb[:, c, m * 128 : (m + 1) * 128],
                rhs=ctxT[:, c, :],
                start=(c == 0),
                stop=(c == EC - 1),
            )

    # ---- b1' = t_emb @ W1 + b1 ----
    b1p = singles.tile([128, C2], F32)
    bps = []
    for m in range(C2):
        bp = psP.tile([128, 1], F32, tag="mega", name=f"bp{m}")
        bps.append(bp)
        for c in range(EC):
            nc.tensor.matmul(
                bp,
                lhsT=w1_sb[:, c, m * 128 : (m + 1) * 128],
                rhs=temb_sb[:, c : c + 1],
                start=(c == 0),
                stop=(c == EC - 1),
            )
        nc.vector.tensor_scalar_add(
            out=b1p[:, m : m + 1], in0=bp, scalar1=b1_sb[:, m : m + 1]
        )

    # ---- silu ----
    hT = singles.tile([128, C2, N], F32)
    xb = singles.tile([128, C2, N], F32)
    sg = singles.tile([128, C2, N], F32)
    for m in range(C2):
        nc.vector.tensor_scalar_add(
            out=xb[:, m, :], in0=hpsum[m], scalar1=b1p[:, m : m + 1]
        )
        nc.scalar.activation(out=sg[:, m, :], in_=xb[:, m, :], func=AFT.Sigmoid)
        nc.vector.tensor_mul(out=hT[:, m, :], in0=xb[:, m, :], in1=sg[:, m, :])

    # ---- dynamic conv weights: wd[d, di, dj*32+n] = w[n, d, di*3+dj] ----
    wd_f = singles.tile([D, 3, 3 * N], F32)
    for di in range(3):
        for dj in range(3):
            k = di * 3 + dj
            wp = psP.tile([D, N], F32, tag="mega")
            for c2 in range(C2):
                w2k = w2_sb[:, c2, :].rearrange("p (d k) -> p d k", k=KK)[:, :, k]
                nc.tensor.matmul(
                    wp,
                    lhsT=w2k,
                    rhs=hT[:, c2, :],
                    start=(c2 == 0),
                    stop=(c2 == C2 - 1),
                )
            nc.vector.tensor_scalar_add(
                out=wd_f[:, di, dj * N : (dj + 1) * N],
                in0=wp,
                scalar1=b2_sb[:, k : k + 1],
            )
    wd = singles.tile([D, 3, 3 * N], F32R)
    nc.sync.dma_start(out=wd, in_=wd_f.bitcast(F32R))

    # ---- latent segments (center only; h-shifts are +/-W col offsets) ----
    # segT[d, c] = latent[d, g0 - MARG + c]
    zstage = singles.tile([D, MARG], F32)
    nc.gpsimd.memset(zstage, 0.0)
    segs = []
    for s in range(NSEG):
        segT = singles.tile([D, SEGW], F32R, name=f"seg{s}")
        segs.append(segT)
        g0 = s * SEGC
        lo, hi = g0 - MARG, g0 + SEGC + MARG
        cl, ch = max(lo, 0), min(hi, HW)
        nc.sync.dma_start(
            out=segT[:, cl - lo : ch - lo], in_=lat[:, cl:ch].bitcast(F32R)
        )
        if cl > lo:
            nc.sync.dma_start(
                out=segT[:, 0 : cl - lo], in_=zstage[:, 0 : cl - lo].bitcast(F32R)
            )
        if hi > ch:
            nc.sync.dma_start(
                out=segT[:, SEGW - (hi - ch) : SEGW],
                in_=zstage[:, 0 : hi - ch].bitcast(F32R),
            )

    # ---- main contraction, groups of GSIZE f-tiles ----
    GW = GSIZE * FT
    for g in range(NG):
        mp = psP.tile([3 * N, GSIZE, FT], F32, tag="mega")
        for t in range(GSIZE):
            f = g * GSIZE + t
            s = f // SEGF
            segT = segs[s]
            cbase = MARG + (f - s * SEGF) * FT
            for di in range(3):
                off = cbase + (di - 1) * W
                nc.tensor.matmul(
                    mp[:, t, :],
                    lhsT=wd[:, di, :],
                    rhs=segT[:, off : off + FT],
                    start=(di == 0),
                    stop=(di == 2),
                )
        ob = outP.tile([N, GW], F32)
        # dj=1 center: one batched copy
        nc.scalar.copy(out=ob, in_=mp[N : 2 * N, :, :])
        o4 = ob.rearrange("p (t h w) -> p t h w", h=FT // W, w=W)
        m4 = mp.rearrange("p t (h w) -> p t h w", w=W)
        # dj=0: out[n,h,w] += tmp0[n,h,w-1] for w>=1
        nc.vector.tensor_add(
            out=o4[:, :, :, 1:W],
            in0=o4[:, :, :, 1:W],
            in1=m4[0:N, :, :, 0 : W - 1],
        )
        # dj=2: out[n,h,w] += tmp2[n,h,w+1] for w<=W-2
        if g % 3 == 0:
            nc.vector.tensor_add(
                out=o4[:, :, :, 0 : W - 1],
                in0=o4[:, :, :, 0 : W - 1],
                in1=m4[2 * N : 3 * N, :, :, 1:W],
            )
        else:
            # offload: ACT stages the raw dj2 group flat to SBUF; GPSIMD
            # applies the +1 column shift in its read AP while adding.
            s2 = outP.tile([N, GW], F32, tag="s2")
            s4 = s2.rearrange("p (t h w) -> p t h w", h=FT // W, w=W)
            nc.scalar.copy(out=s2, in_=mp[2 * N : 3 * N, :, :])
            nc.gpsimd.tensor_add(
                out=o4[:, :, :, 0 : W - 1],
                in0=o4[:, :, :, 0 : W - 1],
                in1=s4[:, :, :, 1:W],
            )
        nc.sync.dma_start(out=out[:, g * GW : (g + 1) * GW], in_=ob)


_NC_CACHE = {}


def _get_nc():
    if "nc" not in _NC_CACHE:
        _NC_CACHE["nc"] = build_nc(B)
    return _NC_CACHE["nc"]


def kernel(latent, t_emb, wv_embs, W1, b1, W2, b2, trace=False, **run_kwargs):
    nc = _get_nc()
    in_maps = []
    for b in range(B):
        in_maps.append(
            {
                "lat": np.ascontiguousarray(latent[b].reshape(D, HW), np.float32),
                "temb": np.ascontiguousarray(t_emb[b], np.float32),
                "wv": np.ascontiguousarray(wv_embs[b], np.float32),
                "w1": np.ascontiguousarray(W1, np.float32),
                "b1": np.ascontiguousarray(b1, np.float32),
                "w2": np.ascontiguousarray(W2, np.float32),
                "b2": np.ascontiguousarray(b2, np.float32),
            }
        )
    try:
        res = run_bass_kernel_spmd(
            nc, in_maps, core_ids=list(range(B)), trace=trace, **run_kwargs
        )
    except ModuleNotFoundError:
        res = run_bass_kernel_spmd(
            nc, in_maps, core_ids=list(range(B)), trace=False, **run_kwargs
        )
    out = np.stack([res.results[b]["out"].reshape(N, H, W) for b in range(B)])
    if trace:
        kernel.last_results = res
    return out

